# revision 5
# baseline (speedup 1.0000x reference)
"""VQ codebook quantizer (Gumbel-softmax) Bass kernel for 8 TRN2 NeuronCores.

Problem: N=16384 tokens, D=1024 dims, G=32 groups, C=256 codes, DSUB=32.
  score[n,g,c] = (-|x|^2 + 2 x.cb - |cb|^2) * exp(-logt[g])
  probs = softmax(score + gumbel, axis=c)       [output 1]
  x_recon = probs @ cb                          [output 2]

Strategy (data-parallel over tokens, 2048/core, 16 tiles of 128 tokens):
  - softmax is shift-invariant: drop the -|x|^2 term (verified no overflow on
    the f32 exponent range; constant K0 recenters), fold 2*exp(-logt) into the
    codebook and -s*|cb|^2 - K0 into a row vector added to gumbel.
  - PE computes 2s*dot via per-group K=32 matmuls (tile_position row packing),
    gumbel + vrow add happens on GPSIMD (vrow) + DVE (PSUM score + gumbel),
    exp on ACT, per-group transposes of exp(u) on PE feed the recon matmul
    whose stationary is u^T (f32r, 12-bit — output-level precision only);
    an appended ones-column yields the softmax row sums for free.
"""
import sys

sys.path.insert(0, "/opt/trn_rl_repo")

import numpy as np
from contextlib import ExitStack

import concourse.bass as bass
import concourse.tile as tile
from concourse import mybir
from concourse.bass_utils import run_bass_kernel_spmd

# ---- problem geometry (hardcoded) ----
N, D, G, C = 16384, 1024, 32, 256
DSUB = D // G          # 32
GC = G * C             # 8192
NCORES = 8
NSH = N // NCORES      # 2048 tokens per core
P = 128                # partition tile
NTILES = NSH // P      # 16
K0 = 12.0              # exponent recentering (max exponent on data ~43)

F32 = mybir.dt.float32
F32R = mybir.dt.float32r

_cache = {}


def split_excess_waits(nc, max_waits=1):
    """Workaround for this toolchain's walrus codegen, which accepts at most
    one sync wait per TPB instruction: hoist excess waits onto same-engine
    Drain instructions inserted immediately before the instruction (engine
    queues execute a block's instructions in order, so this is equivalent)."""
    import bass_rust

    n_split = 0
    for fn in nc.m.functions:
        for bb in fn.blocks:
            insts = bb.instructions
            needs = False
            for i in insts:
                si = i.sync_info
                if si is not None and len(si.on_wait) > max_waits:
                    needs = True
                    break
            if not needs:
                continue
            new = []
            for inst in insts:
                si = inst.sync_info
                if si is not None and len(si.on_wait) > max_waits:
                    waits = list(si.on_wait)
                    extra, keep = waits[:-max_waits], waits[-max_waits:]
                    for k in range(0, len(extra), max_waits):
                        d = mybir.InstDrain(
                            name=nc.get_next_instruction_name(),
                            ins=[], outs=[], bass_is_fusable=False)
                        d.engine = inst.engine
                        d.sync_info = bass_rust.SyncInfo(
                            on_wait=extra[k:k + max_waits], on_update=[])
                        nc.register_instruction(d)
                        new.append(d)
                        n_split += 1
                    si.on_wait = keep
                new.append(inst)
            bb.instructions = new
    return n_split


def _build_program():
    nc = bass.Bass("TRN2", target_bir_lowering=False, debug=False,
                   num_devices=NCORES)

    x_d = nc.dram_tensor("x", [NSH, D], F32, kind="ExternalInput").ap()
    gum_d = nc.dram_tensor("gum", [NSH, GC], F32, kind="ExternalInput").ap()
    cbt_d = nc.dram_tensor("cbt", [128, GC], F32, kind="ExternalInput").ap()
    vrow_d = nc.dram_tensor("vrow", [1, GC], F32, kind="ExternalInput").ap()
    cba_d = nc.dram_tensor("cba", [128, 2176], F32R, kind="ExternalInput").ap()
    id_d = nc.dram_tensor("ident", [128, 128], F32, kind="ExternalInput").ap()
    probs_d = nc.dram_tensor("probs", [NSH, GC], F32, kind="ExternalOutput").ap()
    recon_d = nc.dram_tensor("recon", [NSH, D], F32, kind="ExternalOutput").ap()

    # recon psum layout: 3 banks (512 f32 cols each); 15/15/2 groups of 33 cols
    def recon_col(g):
        if g < 15:
            return 34 * g
        if g < 30:
            return 512 + 34 * (g - 15)
        return 1024 + 34 * (g - 30)

    with tile.TileContext(nc) as tc, ExitStack() as ctx:
        consts = ctx.enter_context(tc.tile_pool(name="consts", bufs=1))
        xpool = ctx.enter_context(tc.tile_pool(name="xpool", bufs=2))
        xtpool = ctx.enter_context(tc.tile_pool(name="xtpool", bufs=2))
        gpool = ctx.enter_context(tc.tile_pool(name="gpool", bufs=2))
        upool = ctx.enter_context(tc.tile_pool(name="upool", bufs=2))
        utpool = ctx.enter_context(tc.tile_pool(name="utpool", bufs=3))
        rpool = ctx.enter_context(tc.tile_pool(name="rpool", bufs=2))
        ps_s = ctx.enter_context(tc.tile_pool(name="ps_s", bufs=3, space="PSUM"))
        ps_t = ctx.enter_context(tc.tile_pool(name="ps_t", bufs=2, space="PSUM"))
        ps_r = ctx.enter_context(tc.tile_pool(name="ps_r", bufs=1, space="PSUM"))

        cbt_sb = consts.tile([128, GC], F32)
        nc.sync.dma_start(cbt_sb[:], cbt_d[:])
        cba_sb = consts.tile([128, 2176], F32R)
        nc.sync.dma_start(cba_sb[:], cba_d[:])
        id_sb = consts.tile([128, 128], F32)
        nc.sync.dma_start(id_sb[:], id_d[:])
        vb_sb = consts.tile([128, GC], F32)
        vb_bcast = bass.AP(
            tensor=vrow_d.tensor, offset=vrow_d.offset,
            ap=[[0, 128]] + [list(p) for p in vrow_d.ap[1:]],
        )
        nc.gpsimd.dma_start(out=vb_sb[:], in_=vb_bcast)

        for t in range(NTILES):
            r0 = t * P
            # ---- x tile in + transpose (8 blocks) ----
            x_tile = xpool.tile([P, D], F32)
            nc.sync.dma_start(x_tile[:], x_d[r0:r0 + P, :])
            xT_sb = xtpool.tile([128, 1024], F32)
            for half in range(2):
                tp = ps_t.tile([128, 512], F32, tag="tp")
                for b4 in range(4):
                    b = 4 * half + b4
                    nc.tensor.transpose(
                        tp[:, 128 * b4:128 * b4 + 128],
                        x_tile[:, 128 * b:128 * b + 128], id_sb[:])
                nc.scalar.copy(xT_sb[:, 512 * half:512 * half + 512], tp[:])

            # ---- gumbel chunks in + vrow pre-add on gpsimd ----
            gch = []
            for q in range(4):
                gt = gpool.tile([P, 2048], F32, tag="gch")
                nc.sync.dma_start(gt[:], gum_d[r0:r0 + P, 2048 * q:2048 * q + 2048])
                nc.gpsimd.tensor_add(gt[:], gt[:], vb_sb[:, 2048 * q:2048 * q + 2048])
                gch.append(gt)

            # ---- scores (16 chunks of 512 cols = 2 groups each) + exp ----
            u_sb = upool.tile([P, GC], F32)
            for c in range(16):
                sp = ps_s.tile([128, 512], F32, tag="sp")
                k = c // 2
                nc.tensor.matmul(
                    sp[:], lhsT=xT_sb[:, 128 * k:128 * k + 128],
                    rhs=cbt_sb[:, 512 * c:512 * c + 512],
                    start=True, stop=True)
                gslice = gch[c // 4][:, 512 * (c % 4):512 * (c % 4) + 512]
                nc.vector.tensor_add(gslice, sp[:], gslice)
                nc.scalar.activation(u_sb[:, 512 * c:512 * c + 512], gslice,
                                     mybir.ActivationFunctionType.Exp)

            # ---- u^T transposes + recon matmuls (16 pairs of 2 groups) ----
            recon_ps = ps_r.tile([128, 1536], F32, tag="recon")
            for gp in range(16):
                up = ps_t.tile([128, 512], F32, tag="tp")
                for ii in range(4):
                    nc.tensor.transpose(
                        up[:, 128 * ii:128 * ii + 128],
                        u_sb[:, 512 * gp + 128 * ii:512 * gp + 128 * ii + 128],
                        id_sb[:])
                uT = utpool.tile([128, 512], F32R, tag="uT")
                nc.scalar.copy(uT[:], up[:])
                for i in range(2):
                    g = 2 * gp + i
                    cg = recon_col(g)
                    for j in range(2):
                        nc.tensor.matmul(
                            recon_ps[:, cg:cg + 34],
                            lhsT=uT[:, 128 * (2 * i + j):128 * (2 * i + j) + 128],
                            rhs=cba_sb[:, 34 * (2 * g + j):34 * (2 * g + j) + 34],
                            start=(j == 0), stop=(j == 1),
                            skip_group_check=True)

            # ---- reciprocal of sums (strided col 32 of each 33-block) ----
            recip = rpool.tile([128, 32], F32, tag="recip")
            for bank, g0, cnt in ((0, 0, 15), (1, 15, 15), (2, 30, 2)):
                sums_ap = recon_ps[:, 512 * bank:512 * bank + 34 * cnt].rearrange(
                    "p (g c) -> p g c", c=34)[:, :, 32]
                nc.vector.reciprocal(recip[:, g0:g0 + cnt], sums_ap)

            # ---- normalize probs in place + DMA out ----
            for g in range(G):
                nc.vector.tensor_scalar_mul(
                    u_sb[:, 256 * g:256 * g + 256],
                    u_sb[:, 256 * g:256 * g + 256], recip[:, g:g + 1])
            for q in range(4):
                nc.sync.dma_start(probs_d[r0:r0 + P, 2048 * q:2048 * q + 2048],
                                  u_sb[:, 2048 * q:2048 * q + 2048])

            # ---- normalize recon + DMA out ----
            recon_sb = rpool.tile([128, 1024], F32, tag="recon_sb")
            for g in range(G):
                cg = recon_col(g)
                nc.vector.tensor_scalar_mul(
                    recon_sb[:, 32 * g:32 * g + 32],
                    recon_ps[:, cg:cg + 32], recip[:, g:g + 1])
            nc.sync.dma_start(recon_d[r0:r0 + P, :], recon_sb[:])

    split_excess_waits(nc)
    return nc


def _host_prep(codebook, log_temperatures):
    s = np.exp(-log_temperatures.astype(np.float64)).astype(np.float32)  # [G]
    ncb = (codebook.astype(np.float64) ** 2).sum(-1).astype(np.float32)  # [G,C]

    # block-diagonal: for d-block k, groups 4k+j sit at rows 32j, cols 256g
    cbt = np.zeros((128, GC), dtype=np.float32)
    for g in range(G):
        k, j = g // 4, g % 4
        cbt[32 * j:32 * j + 32, 256 * g:256 * g + 256] = (
            2.0 * s[g] * codebook[g].T)

    vrow = np.empty((1, GC), dtype=np.float32)
    for g in range(G):
        vrow[0, 256 * g:256 * g + 256] = -s[g] * ncb[g] - K0

    cba = np.zeros((128, 2176), dtype=np.float32)
    for g in range(G):
        for j in range(2):
            blk = cba[:, 34 * (2 * g + j):34 * (2 * g + j) + 34]
            blk[:, :32] = codebook[g, 128 * j:128 * j + 128, :]
            blk[:, 32] = 1.0

    ident = np.eye(128, dtype=np.float32)
    return cbt, vrow, cba, ident


def kernel(x, codebook, log_temperatures, gumbel):
    x = np.ascontiguousarray(np.asarray(x, dtype=np.float32))
    codebook = np.ascontiguousarray(np.asarray(codebook, dtype=np.float32))
    log_temperatures = np.asarray(log_temperatures, dtype=np.float32)
    gumbel = np.ascontiguousarray(np.asarray(gumbel, dtype=np.float32))

    cbt, vrow, cba, ident = _host_prep(codebook, log_temperatures)

    if "nc" not in _cache:
        _cache["nc"] = _build_program()
    nc = _cache["nc"]

    gum2 = gumbel.reshape(N, GC)
    in_maps = []
    for c in range(NCORES):
        sl = slice(c * NSH, (c + 1) * NSH)
        in_maps.append({
            "x": x[sl], "gum": gum2[sl], "cbt": cbt, "vrow": vrow,
            "cba": cba, "ident": ident,
        })

    res = run_bass_kernel_spmd(nc, in_maps, list(range(NCORES))).results

    probs = np.concatenate([r["probs"] for r in res], axis=0).reshape(N, G, C)
    recon = np.concatenate([r["recon"] for r in res], axis=0)
    return recon.astype(np.float32), probs.astype(np.float32)


# revision 6
# speedup vs baseline: 11.9663x; 11.9663x over previous
"""VQ codebook quantizer (Gumbel-softmax) Bass kernel for 8 TRN2 NeuronCores.

Problem: N=16384 tokens, D=1024 dims, G=32 groups, C=256 codes, DSUB=32.
  score[n,g,c] = (-|x|^2 + 2 x.cb - |cb|^2) * exp(-logt[g])
  probs = softmax(score + gumbel, axis=c)       [output 1]
  x_recon = probs @ cb                          [output 2]

Strategy (data-parallel over tokens, 2048/core, 16 tiles of 128 tokens):
  - softmax is shift-invariant: drop the -|x|^2 term (verified no overflow on
    the f32 exponent range; constant K0 recenters), fold 2*exp(-logt) into the
    codebook and -s*|cb|^2 - K0 into a row vector added to gumbel.
  - PE computes 2s*dot via per-group K=32 matmuls (tile_position row packing),
    gumbel + vrow add happens on GPSIMD (vrow) + DVE (PSUM score + gumbel),
    exp on ACT, per-group transposes of exp(u) on PE feed the recon matmul
    whose stationary is u^T (f32r, 12-bit — output-level precision only);
    an appended ones-column yields the softmax row sums for free.
"""
import sys

sys.path.insert(0, "/opt/trn_rl_repo")

import numpy as np
from contextlib import ExitStack

import concourse.bass as bass
import concourse.tile as tile
from concourse import mybir
from concourse.bass_utils import run_bass_kernel_spmd

# ---- problem geometry (hardcoded) ----
N, D, G, C = 16384, 1024, 32, 256
DSUB = D // G          # 32
GC = G * C             # 8192
NCORES = 8
NSH = N // NCORES      # 2048 tokens per core
P = 128                # partition tile
NTILES = NSH // P      # 16
K0 = 12.0              # exponent recentering (max exponent on data ~43)

F32 = mybir.dt.float32
F32R = mybir.dt.float32r

_cache = {}


def split_excess_waits(nc, max_waits=1):
    """Workaround for this toolchain's walrus codegen, which accepts at most
    one sync wait per TPB instruction: hoist excess waits onto same-engine
    Drain instructions inserted immediately before the instruction (engine
    queues execute a block's instructions in order, so this is equivalent)."""
    import bass_rust

    n_split = 0
    for fn in nc.m.functions:
        for bb in fn.blocks:
            insts = bb.instructions
            needs = False
            for i in insts:
                si = i.sync_info
                if si is not None and len(si.on_wait) > max_waits:
                    needs = True
                    break
            if not needs:
                continue
            new = []
            for inst in insts:
                si = inst.sync_info
                if si is not None and len(si.on_wait) > max_waits:
                    waits = list(si.on_wait)
                    extra, keep = waits[:-max_waits], waits[-max_waits:]
                    for k in range(0, len(extra), max_waits):
                        d = mybir.InstDrain(
                            name=nc.get_next_instruction_name(),
                            ins=[], outs=[], bass_is_fusable=False)
                        d.engine = inst.engine
                        d.sync_info = bass_rust.SyncInfo(
                            on_wait=extra[k:k + max_waits], on_update=[])
                        nc.register_instruction(d)
                        new.append(d)
                        n_split += 1
                    si.on_wait = keep
                new.append(inst)
            bb.instructions = new
    return n_split


def _build_program(repeat=1):
    nc = bass.Bass("TRN2", target_bir_lowering=False, debug=False,
                   num_devices=NCORES)

    x_d = nc.dram_tensor("x", [NSH, D], F32, kind="ExternalInput").ap()
    gum_d = nc.dram_tensor("gum", [NSH, GC], F32, kind="ExternalInput").ap()
    cbt_d = nc.dram_tensor("cbt", [128, GC], F32, kind="ExternalInput").ap()
    vrow_d = nc.dram_tensor("vrow", [1, GC], F32, kind="ExternalInput").ap()
    cba_d = nc.dram_tensor("cba", [128, 2176], F32R, kind="ExternalInput").ap()
    id_d = nc.dram_tensor("ident", [128, 128], F32, kind="ExternalInput").ap()
    probs_d = nc.dram_tensor("probs", [NSH, GC], F32, kind="ExternalOutput").ap()
    recon_d = nc.dram_tensor("recon", [NSH, D], F32, kind="ExternalOutput").ap()

    # recon psum layout: 3 banks (512 f32 cols each); 15/15/2 groups of 33 cols
    def recon_col(g):
        if g < 15:
            return 34 * g
        if g < 30:
            return 512 + 34 * (g - 15)
        return 1024 + 34 * (g - 30)

    with tile.TileContext(nc) as tc, ExitStack() as ctx:
        consts = ctx.enter_context(tc.tile_pool(name="consts", bufs=1))
        xpool = ctx.enter_context(tc.tile_pool(name="xpool", bufs=2))
        xtpool = ctx.enter_context(tc.tile_pool(name="xtpool", bufs=2))
        gpool = ctx.enter_context(tc.tile_pool(name="gpool", bufs=2))
        upool = ctx.enter_context(tc.tile_pool(name="upool", bufs=2))
        utpool = ctx.enter_context(tc.tile_pool(name="utpool", bufs=3))
        rpool = ctx.enter_context(tc.tile_pool(name="rpool", bufs=2))
        ps_s = ctx.enter_context(tc.tile_pool(name="ps_s", bufs=3, space="PSUM"))
        ps_t = ctx.enter_context(tc.tile_pool(name="ps_t", bufs=2, space="PSUM"))
        ps_r = ctx.enter_context(tc.tile_pool(name="ps_r", bufs=1, space="PSUM"))

        cbt_sb = consts.tile([128, GC], F32)
        nc.sync.dma_start(cbt_sb[:], cbt_d[:])
        cba_sb = consts.tile([128, 2176], F32R)
        nc.sync.dma_start(cba_sb[:], cba_d[:])
        id_sb = consts.tile([128, 128], F32)
        nc.sync.dma_start(id_sb[:], id_d[:])
        vb_sb = consts.tile([128, GC], F32)
        vb_bcast = bass.AP(
            tensor=vrow_d.tensor, offset=vrow_d.offset,
            ap=[[0, 128]] + [list(p) for p in vrow_d.ap[1:]],
        )
        nc.gpsimd.dma_start(out=vb_sb[:], in_=vb_bcast)

        for t in range(NTILES * repeat):
            r0 = (t % NTILES) * P
            # ---- x tile in + transpose (8 blocks) ----
            x_tile = xpool.tile([P, D], F32)
            nc.sync.dma_start(x_tile[:], x_d[r0:r0 + P, :])
            xT_sb = xtpool.tile([128, 1024], F32)
            for half in range(2):
                tp = ps_t.tile([128, 512], F32, tag="tp")
                for b4 in range(4):
                    b = 4 * half + b4
                    nc.tensor.transpose(
                        tp[:, 128 * b4:128 * b4 + 128],
                        x_tile[:, 128 * b:128 * b + 128], id_sb[:])
                nc.scalar.copy(xT_sb[:, 512 * half:512 * half + 512], tp[:])

            # ---- gumbel chunks in + vrow pre-add on gpsimd ----
            gch = []
            for q in range(4):
                gt = gpool.tile([P, 2048], F32, tag="gch")
                nc.sync.dma_start(gt[:], gum_d[r0:r0 + P, 2048 * q:2048 * q + 2048])
                nc.gpsimd.tensor_add(gt[:], gt[:], vb_sb[:, 2048 * q:2048 * q + 2048])
                gch.append(gt)

            # ---- scores (16 chunks of 512 cols = 2 groups each) + exp ----
            u_sb = upool.tile([P, GC], F32)
            for c in range(16):
                sp = ps_s.tile([128, 512], F32, tag="sp")
                k = c // 2
                nc.tensor.matmul(
                    sp[:], lhsT=xT_sb[:, 128 * k:128 * k + 128],
                    rhs=cbt_sb[:, 512 * c:512 * c + 512],
                    start=True, stop=True)
                gslice = gch[c // 4][:, 512 * (c % 4):512 * (c % 4) + 512]
                nc.vector.tensor_add(gslice, sp[:], gslice)
                nc.scalar.activation(u_sb[:, 512 * c:512 * c + 512], gslice,
                                     mybir.ActivationFunctionType.Exp)

            # ---- u^T transposes + recon matmuls (16 pairs of 2 groups) ----
            recon_ps = ps_r.tile([128, 1536], F32, tag="recon")
            for gp in range(16):
                up = ps_t.tile([128, 512], F32, tag="tp")
                for ii in range(4):
                    nc.tensor.transpose(
                        up[:, 128 * ii:128 * ii + 128],
                        u_sb[:, 512 * gp + 128 * ii:512 * gp + 128 * ii + 128],
                        id_sb[:])
                uT = utpool.tile([128, 512], F32R, tag="uT")
                nc.scalar.copy(uT[:], up[:])
                for i in range(2):
                    g = 2 * gp + i
                    cg = recon_col(g)
                    for j in range(2):
                        nc.tensor.matmul(
                            recon_ps[:, cg:cg + 34],
                            lhsT=uT[:, 128 * (2 * i + j):128 * (2 * i + j) + 128],
                            rhs=cba_sb[:, 34 * (2 * g + j):34 * (2 * g + j) + 34],
                            start=(j == 0), stop=(j == 1),
                            skip_group_check=True)

            # ---- reciprocal of sums (strided col 32 of each 33-block) ----
            recip = rpool.tile([128, 32], F32, tag="recip")
            for bank, g0, cnt in ((0, 0, 15), (1, 15, 15), (2, 30, 2)):
                sums_ap = recon_ps[:, 512 * bank:512 * bank + 34 * cnt].rearrange(
                    "p (g c) -> p g c", c=34)[:, :, 32]
                nc.vector.reciprocal(recip[:, g0:g0 + cnt], sums_ap)

            # ---- normalize probs in place + DMA out ----
            for g in range(G):
                nc.vector.tensor_scalar_mul(
                    u_sb[:, 256 * g:256 * g + 256],
                    u_sb[:, 256 * g:256 * g + 256], recip[:, g:g + 1])
            for q in range(4):
                nc.sync.dma_start(probs_d[r0:r0 + P, 2048 * q:2048 * q + 2048],
                                  u_sb[:, 2048 * q:2048 * q + 2048])

            # ---- normalize recon + DMA out ----
            recon_sb = rpool.tile([128, 1024], F32, tag="recon_sb")
            for g in range(G):
                cg = recon_col(g)
                nc.vector.tensor_scalar_mul(
                    recon_sb[:, 32 * g:32 * g + 32],
                    recon_ps[:, cg:cg + 32], recip[:, g:g + 1])
            nc.sync.dma_start(recon_d[r0:r0 + P, :], recon_sb[:])

    split_excess_waits(nc)
    return nc


def _host_prep(codebook, log_temperatures):
    s = np.exp(-log_temperatures.astype(np.float64)).astype(np.float32)  # [G]
    ncb = (codebook.astype(np.float64) ** 2).sum(-1).astype(np.float32)  # [G,C]

    # block-diagonal: for d-block k, groups 4k+j sit at rows 32j, cols 256g
    cbt = np.zeros((128, GC), dtype=np.float32)
    for g in range(G):
        k, j = g // 4, g % 4
        cbt[32 * j:32 * j + 32, 256 * g:256 * g + 256] = (
            2.0 * s[g] * codebook[g].T)

    vrow = np.empty((1, GC), dtype=np.float32)
    for g in range(G):
        vrow[0, 256 * g:256 * g + 256] = -s[g] * ncb[g] - K0

    cba = np.zeros((128, 2176), dtype=np.float32)
    for g in range(G):
        for j in range(2):
            blk = cba[:, 34 * (2 * g + j):34 * (2 * g + j) + 34]
            blk[:, :32] = codebook[g, 128 * j:128 * j + 128, :]
            blk[:, 32] = 1.0

    ident = np.eye(128, dtype=np.float32)
    return cbt, vrow, cba, ident


def kernel(x, codebook, log_temperatures, gumbel):
    x = np.ascontiguousarray(np.asarray(x, dtype=np.float32))
    codebook = np.ascontiguousarray(np.asarray(codebook, dtype=np.float32))
    log_temperatures = np.asarray(log_temperatures, dtype=np.float32)
    gumbel = np.ascontiguousarray(np.asarray(gumbel, dtype=np.float32))

    cbt, vrow, cba, ident = _host_prep(codebook, log_temperatures)

    if "nc" not in _cache:
        _cache["nc"] = _build_program()
    nc = _cache["nc"]

    gum2 = gumbel.reshape(N, GC)
    in_maps = []
    for c in range(NCORES):
        sl = slice(c * NSH, (c + 1) * NSH)
        in_maps.append({
            "x": x[sl], "gum": gum2[sl], "cbt": cbt, "vrow": vrow,
            "cba": cba, "ident": ident,
        })

    res = run_bass_kernel_spmd(nc, in_maps, list(range(NCORES))).results

    probs = np.concatenate([r["probs"] for r in res], axis=0).reshape(N, G, C)
    recon = np.concatenate([r["recon"] for r in res], axis=0)
    return recon.astype(np.float32), probs.astype(np.float32)
